# revision 6
# baseline (speedup 1.0000x reference)
"""Bahdanau additive attention on TRN2, data-parallel over batch on 8 NeuronCores.

Reference computation (per batch b):
    pre[s, :]  = W1 @ hs[s, b, :] + b1 + W2 @ hidden[b, :] + b2      # (S, H)
    energy[s]  = v . tanh(pre[s, :])                                  # (S,)
    energy     = where(mask[s, b], energy, -1e10)
    attn       = softmax(energy over s)
    ctx[b, :]  = sum_s attn[s] * hs[s, b, :]                          # (H,)

Per-core layout strategy (batch-sharded, 4 batches per core):
  - hsT shard (BL, H, S): h-major so the big matmul streams [h_in=128p, s]
    tiles; preT comes out as [h_out=128p, s] in PSUM, which makes the q/b
    bias a per-partition activation bias and the v-dot a K=128,M=1 matmul
    producing energy in [1, s] (free-axis softmax).
  - hsn shard (S, BL, H): s-major for the context matmul (contract over s).
  All matmuls run as float32r (full fp32 bits, reduced-precision PE mode,
  1 cycle/row at N=512 vs 4 for plain fp32).
"""

import os
import sys
from contextlib import ExitStack

import numpy as np

# Fallback path for concourse; the axon sitecustomize normally provides it.
if "/opt/trn_rl_repo" not in sys.path:
    sys.path.append("/opt/trn_rl_repo")

import concourse.bass as bass
import concourse.bacc as bacc
import concourse.mybir as mybir
import concourse.tile as tile
from concourse import bass_utils

S, B, H = 2048, 32, 1024
NCORES = 8
BL = B // NCORES  # local batches per core
HK = H // 128     # 128-partition chunks of H
NSIG = S // 512   # 512-wide sigma blocks per sequence

F32 = mybir.dt.float32
F32R = mybir.dt.float32r
U8 = mybir.dt.uint8
AF = mybir.ActivationFunctionType
AX = mybir.AxisListType

_CACHE = {}


def _emit(tc, aps):
    nc = tc.nc
    ctx = aps["ctx_stack"]
    hst, hsn, w1t, w2t, hidt, b1r, b2r, vt, masku, ctx_out = (
        aps["hst"], aps["hsn"], aps["w1t"], aps["w2t"], aps["hidt"],
        aps["b1r"], aps["b2r"], aps["vt"], aps["masku"], aps["ctx"],
    )

    def pool(name, bufs, space="SBUF"):
        return ctx.enter_context(tc.tile_pool(name=name, bufs=bufs, space=space))

    p_hst = pool("hst", 3)
    p_w1 = pool("w1", 1)
    p_w2c = pool("w2c", 2)
    p_small = pool("small", 1)
    p_hsn = pool("hsn", 12)
    p_tanh = pool("tanh", 3)
    p_eall = pool("eall", 2)
    p_em = pool("em", 2)
    p_mask = pool("mask", 1)
    p_ctxs = pool("ctxs", 2)
    p_attnT = pool("attnT", 2)
    p_sc = pool("sc", 2)

    pp_pre = pool("ppre", 2, space="PSUM")
    pp_en = pool("pen", 2, space="PSUM")
    pp_tr = pool("ptr", 1, space="PSUM")
    pp_ctx = pool("pctx", 2, space="PSUM")
    pp_q = pool("pq", 1, space="PSUM")

    # ---------------- setup ----------------
    ident = p_small.tile([1, 1], F32, tag="ident")
    nc.gpsimd.memset(ident[:], 1.0)

    b1_sb = p_small.tile([128, HK], F32, tag="b1r")
    nc.sync.dma_start(b1_sb[:], b1r[:])
    b2_sb = p_small.tile([128, HK], F32, tag="b2r")
    nc.sync.dma_start(b2_sb[:], b2r[:])
    vt_sb = p_small.tile([128, HK], F32R, tag="vt")
    nc.sync.dma_start(vt_sb[:], vt[:])
    hid_sb = p_small.tile([128, BL * HK], F32R, tag="hidt")
    for k in range(HK):
        nc.sync.dma_start(hid_sb[:, BL * k:BL * (k + 1)], hidt[128 * k:128 * (k + 1), :])
    mask_sb = []
    for b in range(BL):
        mk = p_mask.tile([1, S], U8, tag="mask", name=f"mask_{b}", bufs=BL)
        nc.sync.dma_start(mk[:], masku[b:b + 1, :])
        mask_sb.append(mk)

    w1_sb = p_w1.tile([128, HK * H], F32R, tag="w1")
    for k in range(HK):
        nc.sync.dma_start(w1_sb[:, H * k:H * (k + 1)], w1t[128 * k:128 * (k + 1), :])

    # qT[h_out, b] = sum_hin W2[h_out, hin] * hidden[b, hin] + b1 + b2
    qt_sb = p_small.tile([128, BL * HK], F32, tag="qt")
    for m in range(HK):
        pq = pp_q.tile([128, BL], F32, tag="pq")
        for k in range(HK):
            w2c = p_w2c.tile([128, 128], F32R, tag="w2c")
            nc.sync.dma_start(w2c[:], w2t[128 * k:128 * (k + 1), 128 * m:128 * (m + 1)])
            nc.tensor.matmul(
                pq[:],
                lhsT=w2c[:],
                rhs=hid_sb[:, BL * k:BL * (k + 1)],
                start=(k == 0), stop=(k == HK - 1),
            )
        nc.vector.tensor_scalar_add(qt_sb[:, BL * m:BL * (m + 1)], pq[:], b1_sb[:, m:m + 1])
        nc.vector.tensor_scalar_add(
            qt_sb[:, BL * m:BL * (m + 1)], qt_sb[:, BL * m:BL * (m + 1)], b2_sb[:, m:m + 1]
        )

    eall_t = {}
    attnT_t = {}
    rz_t = {}

    # ------------- pass 1: energies for one batch -------------
    def p1(b):
        eall = p_eall.tile([1, S], F32, tag="eall")
        eall_t[b] = eall
        for c in range(NSIG):  # 512-wide column blocks of the sequence
            hst_c = p_hst.tile([128, HK * 512], F32R, tag="hst")
            for k in range(HK):
                nc.sync.dma_start(
                    hst_c[:, 512 * k:512 * (k + 1)],
                    hst[b, 128 * k:128 * (k + 1), 512 * c:512 * (c + 1)],
                )
            pen = pp_en.tile([1, 512], F32, tag="pen")
            prev = None
            for m in range(HK):
                ppre = pp_pre.tile([128, 512], F32, tag="ppre")
                for k in range(HK):
                    nc.tensor.matmul(
                        ppre[:],
                        lhsT=w1_sb[:, H * k + 128 * m:H * k + 128 * m + 128],
                        rhs=hst_c[:, 512 * k:512 * (k + 1)],
                        start=(k == 0), stop=(k == HK - 1),
                    )
                # energy matmul for the previous m goes after this m's pre-block
                # so the PE never waits on the tanh.
                if prev is not None:
                    pm, pth = prev
                    nc.tensor.matmul(
                        pen[:],
                        lhsT=vt_sb[:, pm:pm + 1],
                        rhs=pth[:],
                        start=(pm == 0), stop=False,
                    )
                th = p_tanh.tile([128, 512], F32R, tag="tanh")
                nc.scalar.activation(
                    th[:], ppre[:], AF.Tanh,
                    bias=qt_sb[:, BL * m + b:BL * m + b + 1], scale=1.0,
                )
                prev = (m, th)
            pm, pth = prev
            nc.tensor.matmul(
                pen[:],
                lhsT=vt_sb[:, pm:pm + 1],
                rhs=pth[:],
                start=False, stop=True,
            )
            nc.vector.tensor_copy(eall[:, 512 * c:512 * (c + 1)], pen[:])

    # ------------- masked softmax for one batch -------------
    def sm(b):
        eall = eall_t.pop(b)
        em = p_em.tile([1, S], F32, tag="em")
        nc.gpsimd.memset(em[:], -1e10)
        nc.vector.copy_predicated(em[:], mask_sb[b][:], eall[:])
        negmax = p_sc.tile([1, 1], F32, tag="negmax")
        nc.vector.reduce_max(negmax[:], em[:], axis=AX.X, negate=True)
        zs = p_sc.tile([1, 1], F32, tag="zs")
        # attn (unnormalized) = exp(em - max), Z accumulated in the same op
        nc.scalar.activation(em[:], em[:], AF.Exp, bias=negmax[:], scale=1.0, accum_out=zs[:])
        rz = p_sc.tile([1, 1], F32, tag="rz")
        nc.vector.reciprocal(rz[:], zs[:])
        rz_t[b] = rz
        ptr = pp_tr.tile([128, 16], F32, tag="ptr")
        for cc in range(16):
            nc.tensor.transpose(ptr[:, cc:cc + 1], em[:, 128 * cc:128 * (cc + 1)], ident[:])
        att = p_attnT.tile([128, 16], F32R, tag="attnT")
        nc.vector.tensor_copy(att[:], ptr[:])
        attnT_t[b] = att

    # ------------- pass 2: context for one batch -------------
    def p2(b):
        att = attnT_t.pop(b)
        rz = rz_t.pop(b)
        pc = [
            pp_ctx.tile([1, 512], F32, tag="pctx", name=f"pctx_{b}_{n}")
            for n in range(2)
        ]
        for t in range(S // 128):
            hsn_c = p_hsn.tile([128, H], F32R, tag="hsn")
            nc.scalar.dma_start(hsn_c[:], hsn[128 * t:128 * (t + 1), b, :])
            for n in range(2):
                nc.tensor.matmul(
                    pc[n][:],
                    lhsT=att[:, t:t + 1],
                    rhs=hsn_c[:, 512 * n:512 * (n + 1)],
                    start=(t == 0), stop=(t == S // 128 - 1),
                )
        cs = p_ctxs.tile([1, H], F32, tag="ctxs")
        for n in range(2):
            nc.vector.tensor_scalar_mul(cs[:, 512 * n:512 * (n + 1)], pc[n][:], rz[:])
        nc.sync.dma_start(ctx_out[b:b + 1, :], cs[:])

    # ------------- schedule -------------
    p1(0)
    p1(1)
    sm(0)
    p2(0)
    p1(2)
    sm(1)
    p2(1)
    p1(3)
    sm(2)
    p2(2)
    sm(3)
    p2(3)


def build_program():
    if "nc" in _CACHE:
        return _CACHE["nc"]
    nc = bacc.Bacc("TRN2", target_bir_lowering=False, debug=False, enable_asserts=False)
    aps = {
        "hst": nc.dram_tensor("hst", (BL, H, S), F32R, kind="ExternalInput").ap(),
        "hsn": nc.dram_tensor("hsn", (S, BL, H), F32R, kind="ExternalInput").ap(),
        "w1t": nc.dram_tensor("w1t", (H, H), F32R, kind="ExternalInput").ap(),
        "w2t": nc.dram_tensor("w2t", (H, H), F32R, kind="ExternalInput").ap(),
        "hidt": nc.dram_tensor("hidt", (H, BL), F32R, kind="ExternalInput").ap(),
        "b1r": nc.dram_tensor("b1r", (128, HK), F32, kind="ExternalInput").ap(),
        "b2r": nc.dram_tensor("b2r", (128, HK), F32, kind="ExternalInput").ap(),
        "vt": nc.dram_tensor("vt", (128, HK), F32R, kind="ExternalInput").ap(),
        "masku": nc.dram_tensor("masku", (BL, S), U8, kind="ExternalInput").ap(),
        "ctx": nc.dram_tensor("ctx", (BL, H), F32, kind="ExternalOutput").ap(),
    }
    with tile.TileContext(nc) as tc:
        with ExitStack() as stack:
            aps["ctx_stack"] = stack
            _emit(tc, aps)
    nc.compile()
    _CACHE["nc"] = nc
    return nc


def prep_in_maps(inputs):
    hidden = np.ascontiguousarray(np.asarray(inputs["hidden"], dtype=np.float32))
    hs = np.ascontiguousarray(np.asarray(inputs["hidden_sequence"], dtype=np.float32))
    masks = np.asarray(inputs["input_masks"])
    w1t = np.ascontiguousarray(np.asarray(inputs["W1"], dtype=np.float32).T)
    w2t = np.ascontiguousarray(np.asarray(inputs["W2"], dtype=np.float32).T)
    b1 = np.asarray(inputs["b1"], dtype=np.float32)
    b2 = np.asarray(inputs["b2"], dtype=np.float32)
    v = np.asarray(inputs["v"], dtype=np.float32)
    b1r = np.ascontiguousarray(b1.reshape(HK, 128).T)
    b2r = np.ascontiguousarray(b2.reshape(HK, 128).T)
    vt = np.ascontiguousarray(v.reshape(HK, 128).T)
    in_maps = []
    for ci in range(NCORES):
        g = slice(BL * ci, BL * (ci + 1))
        blk = hs[:, g, :]
        in_maps.append({
            "hst": np.ascontiguousarray(blk.transpose(1, 2, 0)),
            "hsn": np.ascontiguousarray(blk),
            "w1t": w1t,
            "w2t": w2t,
            "hidt": np.ascontiguousarray(hidden[0, g, :].T),
            "b1r": b1r,
            "b2r": b2r,
            "vt": vt,
            "masku": np.ascontiguousarray(masks[:, g].T).astype(np.uint8),
        })
    return in_maps


def kernel(**inputs):
    nc = build_program()
    in_maps = prep_in_maps(inputs)
    res = bass_utils.run_bass_kernel_spmd(nc, in_maps, list(range(NCORES)))
    out = np.concatenate([res.results[i]["ctx"] for i in range(NCORES)], axis=0)
    return out[None].astype(np.float32)


if __name__ == "__main__":
    build_program()
    print("program built OK")


# revision 12
# speedup vs baseline: 1.2136x; 1.2136x over previous
"""Bahdanau additive attention on TRN2, data-parallel over batch on 8 NeuronCores.

Reference computation (per batch b):
    pre[s, :]  = W1 @ hs[s, b, :] + b1 + W2 @ hidden[b, :] + b2      # (S, H)
    energy[s]  = v . tanh(pre[s, :])                                  # (S,)
    energy     = where(mask[s, b], energy, -1e10)
    attn       = softmax(energy over s)
    ctx[b, :]  = sum_s attn[s] * hs[s, b, :]                          # (H,)

Per-core layout strategy (batch-sharded, 4 batches per core):
  - hsT shard (BL, H, S): h-major so the big matmul streams [h_in=128p, s]
    tiles; preT comes out as [h_out=128p, s] in PSUM, which makes the q/b
    bias a per-partition activation bias and the v-dot a K=128,M=1 matmul
    producing energy in [1, s] (free-axis softmax).
  - hsn shard (S, BL, H): s-major for the context matmul (contract over s).
  All matmuls run as float32r (full fp32 bits, reduced-precision PE mode,
  1 cycle/row at N=512 vs 4 for plain fp32).
"""

import os
import sys
from contextlib import ExitStack

import numpy as np

# Fallback path for concourse; the axon sitecustomize normally provides it.
if "/opt/trn_rl_repo" not in sys.path:
    sys.path.append("/opt/trn_rl_repo")

import concourse.bass as bass
import concourse.bacc as bacc
import concourse.mybir as mybir
import concourse.tile as tile
from concourse import bass_utils

S, B, H = 2048, 32, 1024
NCORES = 8
BL = B // NCORES  # local batches per core
HK = H // 128     # 128-partition chunks of H
NSIG = S // 512   # 512-wide sigma blocks per sequence

F32 = mybir.dt.float32
F32R = mybir.dt.float32r
U8 = mybir.dt.uint8
AF = mybir.ActivationFunctionType
AX = mybir.AxisListType

_CACHE = {}


def _emit(tc, aps):
    nc = tc.nc
    ctx = aps["ctx_stack"]
    hst, hsn, w1t, w2t, hidt, b1r, b2r, vt, masku, ctx_out = (
        aps["hst"], aps["hsn"], aps["w1t"], aps["w2t"], aps["hidt"],
        aps["b1r"], aps["b2r"], aps["vt"], aps["masku"], aps["ctx"],
    )

    def pool(name, bufs, space="SBUF"):
        return ctx.enter_context(tc.tile_pool(name=name, bufs=bufs, space=space))

    p_hst = pool("hst", 3)
    p_w1 = pool("w1", 1)
    p_w2c = pool("w2c", 2)
    p_small = pool("small", 1)
    p_hsn = pool("hsn", 10)
    p_tanh = pool("tanh", 3)
    p_eall = pool("eall", 2)
    p_em = pool("em", 2)
    p_mask = pool("mask", 1)
    p_ctxs = pool("ctxs", 2)
    p_attnT = pool("attnT", 2)
    p_sc = pool("sc", 2)

    pp_pre = pool("ppre", 3, space="PSUM")
    pp_en = pool("pen", 1, space="PSUM")
    pp_tr = pool("ptr", 1, space="PSUM")
    pp_ctx = pool("pctx", 2, space="PSUM")
    pp_q = pool("pq", 1, space="PSUM")

    # ---------------- setup ----------------
    ident = p_small.tile([1, 1], F32, tag="ident")
    nc.gpsimd.memset(ident[:], 1.0)

    b1_sb = p_small.tile([128, HK], F32, tag="b1r")
    nc.sync.dma_start(b1_sb[:], b1r[:])
    b2_sb = p_small.tile([128, HK], F32, tag="b2r")
    nc.sync.dma_start(b2_sb[:], b2r[:])
    vt_sb = p_small.tile([128, HK], F32R, tag="vt")
    nc.sync.dma_start(vt_sb[:], vt[:])
    hid_sb = p_small.tile([128, BL * HK], F32R, tag="hidt")
    for k in range(HK):
        nc.sync.dma_start(hid_sb[:, BL * k:BL * (k + 1)], hidt[128 * k:128 * (k + 1), :])
    mask_sb = []
    for b in range(BL):
        mk = p_mask.tile([1, S], U8, tag="mask", name=f"mask_{b}", bufs=BL)
        nc.sync.dma_start(mk[:], masku[b:b + 1, :])
        mask_sb.append(mk)

    w1_sb = p_w1.tile([128, HK * H], F32R, tag="w1")
    for k in range(HK):
        nc.sync.dma_start(w1_sb[:, H * k:H * (k + 1)], w1t[128 * k:128 * (k + 1), :])

    # qT[h_out, b] = sum_hin W2[h_out, hin] * hidden[b, hin] + b1 + b2
    # k-outer with one big DMA per k-row-block of W2T; all 8 m-accumulation
    # groups live in one PSUM bank (interleaved groups are element-wise safe).
    qt_sb = p_small.tile([128, BL * HK], F32, tag="qt")
    pq_all = pp_q.tile([128, BL * HK], F32, tag="pq")
    for k in range(HK):
        w2r = p_w2c.tile([128, H], F32R, tag="w2c", name=f"w2r{k}")
        nc.sync.dma_start(w2r[:], w2t[128 * k:128 * (k + 1), :])
        for m in range(HK):
            nc.tensor.matmul(
                pq_all[:, BL * m:BL * (m + 1)],
                lhsT=w2r[:, 128 * m:128 * (m + 1)],
                rhs=hid_sb[:, BL * k:BL * (k + 1)],
                start=(k == 0 and m == 0), stop=(k == HK - 1 and m == HK - 1),
                skip_group_check=True,
            )
    for m in range(HK):
        nc.vector.tensor_scalar_add(
            qt_sb[:, BL * m:BL * (m + 1)], pq_all[:, BL * m:BL * (m + 1)], b1_sb[:, m:m + 1]
        )
        nc.vector.tensor_scalar_add(
            qt_sb[:, BL * m:BL * (m + 1)], qt_sb[:, BL * m:BL * (m + 1)], b2_sb[:, m:m + 1]
        )

    eall_t = {}
    attnT_t = {}
    rz_t = {}

    # ------------- pass 1: energies for one batch -------------
    def p1(b):
        eall = p_eall.tile([1, S], F32, tag="eall")
        eall_t[b] = eall
        for c in range(NSIG):  # 512-wide column blocks of the sequence
            hst_c = p_hst.tile([128, HK * 512], F32R, tag="hst")
            for k in range(HK):
                nc.sync.dma_start(
                    hst_c[:, 512 * k:512 * (k + 1)],
                    hst[b, 128 * k:128 * (k + 1), 512 * c:512 * (c + 1)],
                )
            pen = pp_en.tile([1, 512], F32, tag="pen")
            prev = None
            for m in range(HK):
                ppre = pp_pre.tile([128, 512], F32, tag="ppre")
                for k in range(HK):
                    nc.tensor.matmul(
                        ppre[:],
                        lhsT=w1_sb[:, H * k + 128 * m:H * k + 128 * m + 128],
                        rhs=hst_c[:, 512 * k:512 * (k + 1)],
                        start=(k == 0), stop=(k == HK - 1),
                    )
                # energy matmul for the previous m goes after this m's pre-block
                # so the PE never waits on the tanh.
                if prev is not None:
                    pm, pth = prev
                    nc.tensor.matmul(
                        pen[:],
                        lhsT=vt_sb[:, pm:pm + 1],
                        rhs=pth[:],
                        start=(pm == 0), stop=False,
                    )
                th = p_tanh.tile([128, 512], F32R, tag="tanh")
                nc.scalar.activation(
                    th[:], ppre[:], AF.Tanh,
                    bias=qt_sb[:, BL * m + b:BL * m + b + 1], scale=1.0,
                )
                prev = (m, th)
            pm, pth = prev
            nc.tensor.matmul(
                pen[:],
                lhsT=vt_sb[:, pm:pm + 1],
                rhs=pth[:],
                start=False, stop=True,
            )
            nc.vector.tensor_copy(eall[:, 512 * c:512 * (c + 1)], pen[:])

    # ------------- masked softmax for one batch -------------
    def sm(b):
        eall = eall_t.pop(b)
        em = p_em.tile([1, S], F32, tag="em")
        # em = -1e10 everywhere (eall*0 - 1e10 keeps this on the DVE), then
        # unmasked positions are overwritten with the real energies.
        nc.vector.tensor_scalar(
            em[:], eall[:], 0.0, -1e10,
            op0=mybir.AluOpType.mult, op1=mybir.AluOpType.add,
        )
        nc.vector.copy_predicated(em[:], mask_sb[b][:], eall[:])
        negmax = p_sc.tile([1, 1], F32, tag="negmax")
        nc.vector.reduce_max(negmax[:], em[:], axis=AX.X, negate=True)
        zs = p_sc.tile([1, 1], F32, tag="zs")
        # attn (unnormalized) = exp(em - max), Z accumulated in the same op
        nc.scalar.activation(em[:], em[:], AF.Exp, bias=negmax[:], scale=1.0, accum_out=zs[:])
        rz = p_sc.tile([1, 1], F32, tag="rz")
        nc.vector.reciprocal(rz[:], zs[:])
        rz_t[b] = rz
        ptr = pp_tr.tile([128, 16], F32, tag="ptr")
        for cc in range(16):
            nc.tensor.transpose(ptr[:, cc:cc + 1], em[:, 128 * cc:128 * (cc + 1)], ident[:])
        att = p_attnT.tile([128, 16], F32R, tag="attnT")
        nc.vector.tensor_copy(att[:], ptr[:])
        attnT_t[b] = att

    # ------------- pass 2: context for one batch -------------
    def p2(b):
        att = attnT_t.pop(b)
        rz = rz_t.pop(b)
        pc = [
            pp_ctx.tile([1, 512], F32, tag="pctx", name=f"pctx_{b}_{n}")
            for n in range(2)
        ]
        for t in range(S // 128):
            hsn_c = p_hsn.tile([128, H], F32R, tag="hsn")
            nc.gpsimd.dma_start(hsn_c[:], hsn[128 * t:128 * (t + 1), b, :])
            for n in range(2):
                nc.tensor.matmul(
                    pc[n][:],
                    lhsT=att[:, t:t + 1],
                    rhs=hsn_c[:, 512 * n:512 * (n + 1)],
                    start=(t == 0), stop=(t == S // 128 - 1),
                )
        cs = p_ctxs.tile([1, H], F32, tag="ctxs")
        for n in range(2):
            nc.vector.tensor_scalar_mul(cs[:, 512 * n:512 * (n + 1)], pc[n][:], rz[:])
        nc.sync.dma_start(ctx_out[b:b + 1, :], cs[:])

    # ------------- schedule -------------
    p1(0)
    p1(1)
    sm(0)
    p2(0)
    p1(2)
    sm(1)
    p2(1)
    p1(3)
    sm(2)
    p2(2)
    sm(3)
    p2(3)


def build_program():
    if "nc" in _CACHE:
        return _CACHE["nc"]
    nc = bacc.Bacc("TRN2", target_bir_lowering=False, debug=False, enable_asserts=False)
    aps = {
        "hst": nc.dram_tensor("hst", (BL, H, S), F32R, kind="ExternalInput").ap(),
        "hsn": nc.dram_tensor("hsn", (S, BL, H), F32R, kind="ExternalInput").ap(),
        "w1t": nc.dram_tensor("w1t", (H, H), F32R, kind="ExternalInput").ap(),
        "w2t": nc.dram_tensor("w2t", (H, H), F32R, kind="ExternalInput").ap(),
        "hidt": nc.dram_tensor("hidt", (H, BL), F32R, kind="ExternalInput").ap(),
        "b1r": nc.dram_tensor("b1r", (128, HK), F32, kind="ExternalInput").ap(),
        "b2r": nc.dram_tensor("b2r", (128, HK), F32, kind="ExternalInput").ap(),
        "vt": nc.dram_tensor("vt", (128, HK), F32R, kind="ExternalInput").ap(),
        "masku": nc.dram_tensor("masku", (BL, S), U8, kind="ExternalInput").ap(),
        "ctx": nc.dram_tensor("ctx", (BL, H), F32, kind="ExternalOutput").ap(),
    }
    with tile.TileContext(nc) as tc:
        with ExitStack() as stack:
            aps["ctx_stack"] = stack
            _emit(tc, aps)
    nc.compile()
    _CACHE["nc"] = nc
    return nc


def prep_in_maps(inputs):
    hidden = np.ascontiguousarray(np.asarray(inputs["hidden"], dtype=np.float32))
    hs = np.ascontiguousarray(np.asarray(inputs["hidden_sequence"], dtype=np.float32))
    masks = np.asarray(inputs["input_masks"])
    w1t = np.ascontiguousarray(np.asarray(inputs["W1"], dtype=np.float32).T)
    w2t = np.ascontiguousarray(np.asarray(inputs["W2"], dtype=np.float32).T)
    b1 = np.asarray(inputs["b1"], dtype=np.float32)
    b2 = np.asarray(inputs["b2"], dtype=np.float32)
    v = np.asarray(inputs["v"], dtype=np.float32)
    b1r = np.ascontiguousarray(b1.reshape(HK, 128).T)
    b2r = np.ascontiguousarray(b2.reshape(HK, 128).T)
    vt = np.ascontiguousarray(v.reshape(HK, 128).T)
    in_maps = []
    for ci in range(NCORES):
        g = slice(BL * ci, BL * (ci + 1))
        blk = hs[:, g, :]
        in_maps.append({
            "hst": np.ascontiguousarray(blk.transpose(1, 2, 0)),
            "hsn": np.ascontiguousarray(blk),
            "w1t": w1t,
            "w2t": w2t,
            "hidt": np.ascontiguousarray(hidden[0, g, :].T),
            "b1r": b1r,
            "b2r": b2r,
            "vt": vt,
            "masku": np.ascontiguousarray(masks[:, g].T).astype(np.uint8),
        })
    return in_maps


def kernel(**inputs):
    nc = build_program()
    in_maps = prep_in_maps(inputs)
    res = bass_utils.run_bass_kernel_spmd(nc, in_maps, list(range(NCORES)))
    out = np.concatenate([res.results[i]["ctx"] for i in range(NCORES)], axis=0)
    return out[None].astype(np.float32)


if __name__ == "__main__":
    build_program()
    print("program built OK")


# revision 16
# speedup vs baseline: 1.2314x; 1.0147x over previous
"""Bahdanau additive attention on TRN2, data-parallel over batch on 8 NeuronCores.

Reference computation (per batch b):
    pre[s, :]  = W1 @ hs[s, b, :] + b1 + W2 @ hidden[b, :] + b2      # (S, H)
    energy[s]  = v . tanh(pre[s, :])                                  # (S,)
    energy     = where(mask[s, b], energy, -1e10)
    attn       = softmax(energy over s)
    ctx[b, :]  = sum_s attn[s] * hs[s, b, :]                          # (H,)

Per-core layout strategy (batch-sharded, 4 batches per core):
  - hsT shard (BL, H, S): h-major so the big matmul streams [h_in=128p, s]
    tiles; preT comes out as [h_out=128p, s] in PSUM, which makes the q/b
    bias a per-partition activation bias and the v-dot a K=128,M=1 matmul
    producing energy in [1, s] (free-axis softmax).
  - hsn shard (S, BL, H): s-major for the context matmul (contract over s).
  All matmuls run as float32r (full fp32 bits, reduced-precision PE mode,
  1 cycle/row at N=512 vs 4 for plain fp32).
"""

import os
import sys
from contextlib import ExitStack

import numpy as np

# Fallback path for concourse; the axon sitecustomize normally provides it.
if "/opt/trn_rl_repo" not in sys.path:
    sys.path.append("/opt/trn_rl_repo")

import concourse.bass as bass
import concourse.bacc as bacc
import concourse.mybir as mybir
import concourse.tile as tile
from concourse import bass_utils

S, B, H = 2048, 32, 1024
NCORES = 8
BL = B // NCORES  # local batches per core
HK = H // 128     # 128-partition chunks of H
NSIG = S // 512   # 512-wide sigma blocks per sequence

F32 = mybir.dt.float32
F32R = mybir.dt.float32r
U8 = mybir.dt.uint8
AF = mybir.ActivationFunctionType
AX = mybir.AxisListType

_CACHE = {}


def _emit(tc, aps):
    nc = tc.nc
    ctx = aps["ctx_stack"]
    hst, hsn, w1t, w2t, hidt, b1r, b2r, vt, masku, ctx_out = (
        aps["hst"], aps["hsn"], aps["w1t"], aps["w2t"], aps["hidt"],
        aps["b1r"], aps["b2r"], aps["vt"], aps["masku"], aps["ctx"],
    )

    def pool(name, bufs, space="SBUF"):
        return ctx.enter_context(tc.tile_pool(name=name, bufs=bufs, space=space))

    p_hst = pool("hst", 3)
    p_w1 = pool("w1", 1)
    p_w2c = pool("w2c", 2)
    p_small = pool("small", 1)
    p_hsn = pool("hsn", 10)
    p_tanh = pool("tanh", 3)
    p_eall = pool("eall", 2)
    p_em = pool("em", 2)
    p_mask = pool("mask", 1)
    p_ctxs = pool("ctxs", 2)
    p_attnT = pool("attnT", 2)
    p_sc = pool("sc", 2)

    pp_pre = pool("ppre", 3, space="PSUM")
    pp_en = pool("pen", 1, space="PSUM")
    pp_tr = pool("ptr", 1, space="PSUM")
    pp_ctx = pool("pctx", 2, space="PSUM")
    pp_q = pool("pq", 1, space="PSUM")

    # ---------------- setup ----------------
    ident = p_small.tile([1, 1], F32, tag="ident")
    nc.gpsimd.memset(ident[:], 1.0)

    b1_sb = p_small.tile([128, HK], F32, tag="b1r")
    nc.sync.dma_start(b1_sb[:], b1r[:])
    b2_sb = p_small.tile([128, HK], F32, tag="b2r")
    nc.sync.dma_start(b2_sb[:], b2r[:])
    vt_sb = p_small.tile([128, HK], F32R, tag="vt")
    nc.sync.dma_start(vt_sb[:], vt[:])
    hid_sb = p_small.tile([128, BL * HK], F32R, tag="hidt")
    for k in range(HK):
        nc.sync.dma_start(hid_sb[:, BL * k:BL * (k + 1)], hidt[128 * k:128 * (k + 1), :])
    mask_sb = []
    for b in range(BL):
        mk = p_mask.tile([1, S], U8, tag="mask", name=f"mask_{b}", bufs=BL)
        nc.sync.dma_start(mk[:], masku[b:b + 1, :])
        mask_sb.append(mk)

    # qT[h_out, b] = sum_hin W2[h_out, hin] * hidden[b, hin] + b1 + b2.
    # One PSUM accumulation group spans all 64 matmuls (zero-region marking is
    # bank-granular, so per-m groups would clobber each other). Runs first so
    # the PE computes q while W1T / the first hsT block are still streaming.
    qt_sb = p_small.tile([128, BL * HK], F32, tag="qt")
    pq_all = pp_q.tile([128, BL * HK], F32, tag="pq")
    for k in range(HK):
        w2r = p_w2c.tile([128, H], F32R, tag="w2c", name=f"w2r{k}")
        nc.sync.dma_start(w2r[:], w2t[128 * k:128 * (k + 1), :])
        for m in range(HK):
            nc.tensor.matmul(
                pq_all[:, BL * m:BL * (m + 1)],
                lhsT=w2r[:, 128 * m:128 * (m + 1)],
                rhs=hid_sb[:, BL * k:BL * (k + 1)],
                start=(k == 0 and m == 0), stop=(k == HK - 1 and m == HK - 1),
                skip_group_check=True,
            )
    for m in range(HK):
        nc.vector.tensor_scalar_add(
            qt_sb[:, BL * m:BL * (m + 1)], pq_all[:, BL * m:BL * (m + 1)], b1_sb[:, m:m + 1]
        )
        nc.vector.tensor_scalar_add(
            qt_sb[:, BL * m:BL * (m + 1)], qt_sb[:, BL * m:BL * (m + 1)], b2_sb[:, m:m + 1]
        )

    # W1T and the first hsT block land k-interleaved behind the W2 rows.
    w1_sb = p_w1.tile([128, HK * H], F32R, tag="w1")
    hst_first = p_hst.tile([128, HK * 512], F32R, tag="hst", name="hst_first")
    for k in range(HK):
        nc.sync.dma_start(w1_sb[:, H * k:H * (k + 1)], w1t[128 * k:128 * (k + 1), :])
        nc.sync.dma_start(hst_first[:, 512 * k:512 * (k + 1)], hst[0, 128 * k:128 * (k + 1), 0:512])

    eall_t = {}
    attnT_t = {}
    rz_t = {}

    # ------------- pass 1: energies for one batch -------------
    def p1(b, first_tile=None, inject=None):
        eall = p_eall.tile([1, S], F32, tag="eall")
        eall_t[b] = eall
        for c in range(NSIG):  # 512-wide column blocks of the sequence
            if c == 0 and first_tile is not None:
                hst_c = first_tile
            else:
                hst_c = p_hst.tile([128, HK * 512], F32R, tag="hst")
                for k in range(HK):
                    nc.sync.dma_start(
                        hst_c[:, 512 * k:512 * (k + 1)],
                        hst[b, 128 * k:128 * (k + 1), 512 * c:512 * (c + 1)],
                    )
            pen = pp_en.tile([1, 512], F32, tag="pen")
            prev = None
            for m in range(HK):
                ppre = pp_pre.tile([128, 512], F32, tag="ppre")
                for k in range(HK):
                    nc.tensor.matmul(
                        ppre[:],
                        lhsT=w1_sb[:, H * k + 128 * m:H * k + 128 * m + 128],
                        rhs=hst_c[:, 512 * k:512 * (k + 1)],
                        start=(k == 0), stop=(k == HK - 1),
                    )
                # energy matmul for the previous m goes after this m's pre-block
                # so the PE never waits on the tanh.
                if prev is not None:
                    pm, pth = prev
                    nc.tensor.matmul(
                        pen[:],
                        lhsT=vt_sb[:, pm:pm + 1],
                        rhs=pth[:],
                        start=(pm == 0), stop=False,
                    )
                th = p_tanh.tile([128, 512], F32R, tag="tanh")
                nc.scalar.activation(
                    th[:], ppre[:], AF.Tanh,
                    bias=qt_sb[:, BL * m + b:BL * m + b + 1], scale=1.0,
                )
                prev = (m, th)
            pm, pth = prev
            nc.tensor.matmul(
                pen[:],
                lhsT=vt_sb[:, pm:pm + 1],
                rhs=pth[:],
                start=False, stop=True,
            )
            nc.vector.tensor_copy(eall[:, 512 * c:512 * (c + 1)], pen[:])

    # ------------- masked softmax for one batch -------------
    def sm(b):
        eall = eall_t.pop(b)
        em = p_em.tile([1, S], F32, tag="em")
        # em = -1e10 everywhere (eall*0 - 1e10 keeps this on the DVE), then
        # unmasked positions are overwritten with the real energies.
        nc.vector.tensor_scalar(
            em[:], eall[:], 0.0, -1e10,
            op0=mybir.AluOpType.mult, op1=mybir.AluOpType.add,
        )
        nc.vector.copy_predicated(em[:], mask_sb[b][:], eall[:])
        negmax = p_sc.tile([1, 1], F32, tag="negmax")
        nc.vector.reduce_max(negmax[:], em[:], axis=AX.X, negate=True)
        zs = p_sc.tile([1, 1], F32, tag="zs")
        # attn (unnormalized) = exp(em - max), Z accumulated in the same op
        nc.scalar.activation(em[:], em[:], AF.Exp, bias=negmax[:], scale=1.0, accum_out=zs[:])
        rz = p_sc.tile([1, 1], F32, tag="rz")
        nc.vector.reciprocal(rz[:], zs[:])
        rz_t[b] = rz
        ptr = pp_tr.tile([128, 16], F32, tag="ptr")
        for cc in range(16):
            nc.tensor.transpose(ptr[:, cc:cc + 1], em[:, 128 * cc:128 * (cc + 1)], ident[:])
        att = p_attnT.tile([128, 16], F32R, tag="attnT")
        nc.vector.tensor_copy(att[:], ptr[:])
        attnT_t[b] = att

    # ------------- pass 2: context for one batch -------------
    hsn_tiles = {}

    def p2_load(b):
        tiles = []
        for t in range(S // 128):
            hsn_c = p_hsn.tile([128, H], F32R, tag="hsn", name=f"hsn_{b}_{t}")
            nc.gpsimd.dma_start(hsn_c[:], hsn[128 * t:128 * (t + 1), b, :])
            tiles.append(hsn_c)
        hsn_tiles[b] = tiles

    def p2_mm(b):
        att = attnT_t.pop(b)
        rz = rz_t.pop(b)
        pc = [
            pp_ctx.tile([1, 512], F32, tag="pctx", name=f"pctx_{b}_{n}")
            for n in range(2)
        ]
        for t, hsn_c in enumerate(hsn_tiles.pop(b)):
            for n in range(2):
                nc.tensor.matmul(
                    pc[n][:],
                    lhsT=att[:, t:t + 1],
                    rhs=hsn_c[:, 512 * n:512 * (n + 1)],
                    start=(t == 0), stop=(t == S // 128 - 1),
                )
        cs = p_ctxs.tile([1, H], F32, tag="ctxs")
        for n in range(2):
            nc.vector.tensor_scalar_mul(cs[:, 512 * n:512 * (n + 1)], pc[n][:], rz[:])
        nc.sync.dma_start(ctx_out[b:b + 1, :], cs[:])

    # ------------- schedule -------------
    p1(0, first_tile=hst_first)
    p2_load(0)
    p1(1)
    sm(0)
    p2_mm(0)
    p1(2)
    p2_load(1)
    sm(1)
    p2_mm(1)
    p1(3)
    p2_load(2)
    sm(2)
    p2_mm(2)
    p2_load(3)
    sm(3)
    p2_mm(3)


def build_program():
    if "nc" in _CACHE:
        return _CACHE["nc"]
    nc = bacc.Bacc("TRN2", target_bir_lowering=False, debug=False, enable_asserts=False)
    aps = {
        "hst": nc.dram_tensor("hst", (BL, H, S), F32R, kind="ExternalInput").ap(),
        "hsn": nc.dram_tensor("hsn", (S, BL, H), F32R, kind="ExternalInput").ap(),
        "w1t": nc.dram_tensor("w1t", (H, H), F32R, kind="ExternalInput").ap(),
        "w2t": nc.dram_tensor("w2t", (H, H), F32R, kind="ExternalInput").ap(),
        "hidt": nc.dram_tensor("hidt", (H, BL), F32R, kind="ExternalInput").ap(),
        "b1r": nc.dram_tensor("b1r", (128, HK), F32, kind="ExternalInput").ap(),
        "b2r": nc.dram_tensor("b2r", (128, HK), F32, kind="ExternalInput").ap(),
        "vt": nc.dram_tensor("vt", (128, HK), F32R, kind="ExternalInput").ap(),
        "masku": nc.dram_tensor("masku", (BL, S), U8, kind="ExternalInput").ap(),
        "ctx": nc.dram_tensor("ctx", (BL, H), F32, kind="ExternalOutput").ap(),
    }
    with tile.TileContext(nc) as tc:
        with ExitStack() as stack:
            aps["ctx_stack"] = stack
            _emit(tc, aps)
    nc.compile()
    _CACHE["nc"] = nc
    return nc


def prep_in_maps(inputs):
    hidden = np.ascontiguousarray(np.asarray(inputs["hidden"], dtype=np.float32))
    hs = np.ascontiguousarray(np.asarray(inputs["hidden_sequence"], dtype=np.float32))
    masks = np.asarray(inputs["input_masks"])
    w1t = np.ascontiguousarray(np.asarray(inputs["W1"], dtype=np.float32).T)
    w2t = np.ascontiguousarray(np.asarray(inputs["W2"], dtype=np.float32).T)
    b1 = np.asarray(inputs["b1"], dtype=np.float32)
    b2 = np.asarray(inputs["b2"], dtype=np.float32)
    v = np.asarray(inputs["v"], dtype=np.float32)
    b1r = np.ascontiguousarray(b1.reshape(HK, 128).T)
    b2r = np.ascontiguousarray(b2.reshape(HK, 128).T)
    vt = np.ascontiguousarray(v.reshape(HK, 128).T)
    in_maps = []
    for ci in range(NCORES):
        g = slice(BL * ci, BL * (ci + 1))
        blk = hs[:, g, :]
        in_maps.append({
            "hst": np.ascontiguousarray(blk.transpose(1, 2, 0)),
            "hsn": np.ascontiguousarray(blk),
            "w1t": w1t,
            "w2t": w2t,
            "hidt": np.ascontiguousarray(hidden[0, g, :].T),
            "b1r": b1r,
            "b2r": b2r,
            "vt": vt,
            "masku": np.ascontiguousarray(masks[:, g].T).astype(np.uint8),
        })
    return in_maps


def kernel(**inputs):
    nc = build_program()
    in_maps = prep_in_maps(inputs)
    res = bass_utils.run_bass_kernel_spmd(nc, in_maps, list(range(NCORES)))
    out = np.concatenate([res.results[i]["ctx"] for i in range(NCORES)], axis=0)
    return out[None].astype(np.float32)


if __name__ == "__main__":
    build_program()
    print("program built OK")
